# revision 39
# baseline (speedup 1.0000x reference)
"""AHGCRU (hypergraph-conv GRU) Trainium2 kernel.

Data-parallel over batch: B=16 -> 2 batch elements per NeuronCore (8 cores),
graph/params replicated, no collectives.

Host precompute collapses the N->M->N hypergraph aggregation into one dense
(N,N) matrix A2 = diag(Dinv) @ adj @ diag(Binv) @ S.T  (adjacency depends only
on nodevec/edgevec, so it is identical for every batch element and timestep),
then rank-128-truncates it via SVD (A2 ~= Ur @ Vr): the spectral tail is far
below the bf16 noise floor, and the factored aggregation
pre += Ur @ (Vr @ hl) costs 4x fewer PE columns than the dense product.

Device layout: channels-on-partitions, nodes on the free dim, both local batch
elements side by side: (C, 2048) = [b0 nodes 0:1024 | b1 nodes 1024:2048]
(N=1000 padded to 1024).  LayerNorm over channels is done with PE assists:
  - mean-centering via a projection-matrix GEMM  P = diag(gamma) @ (I - 1/C)
  - variance via a weighted ones-matrix GEMM     G[k,m] = (1/C)/gamma_k^2
  - rstd = exp(-0.5*ln(var+eps)) on ScalarE (ln/exp share one act table).
Sigmoid/tanh are computed as exp (ScalarE, same natural_log_exp table -> no
ACT_TABLE_LOAD per step) + fast custom-DVE reciprocal; relu runs on DVE
(tensor_scalar_max) so the ScalarE chain is ln/exp/square/copies only.
Every PSUM tile is one [128,512] bank keyed per (batch, half) so chain
stages pipeline tile-granularly; agg banks are emitted interleaved with the
previous tile's LN chain to keep PE continuously busy (p-state ramp).
State channels sit at partitions 0:64, a constant ones-row at 64 (also the
bias row of the bias-augmented 1x1 conv), and x_t channels at 65:97.
"""

import os
import sys

import numpy as np

for _p in ("/opt/trn_rl_repo", "/opt/pypackages"):
    if os.path.isdir(_p) and _p not in sys.path:
        sys.path.insert(0, _p)

B, N, F_IN, T = 16, 1000, 32, 12
HID = 64
OUT = 64
M = 500
EMB = 16
EPS = 1e-5

NCORES = 8
BL = B // NCORES          # 2 batch elements per core
NP = 1024                 # padded node count
NC = BL * NP              # 2048 free columns
NCHUNK = NP // 128        # 8 source chunks of 128 nodes per batch element


# --------------------------------------------------------------------------
# host-side preprocessing
# --------------------------------------------------------------------------

def _to_bf16(a):
    import ml_dtypes  # noqa: PLC0415

    return np.asarray(a, dtype=np.float32).astype(ml_dtypes.bfloat16)


def _host_prep(inputs):
    """Build all device-side constant tensors (shared across cores)."""
    f64 = np.float64
    nodevec = inputs["nodevec"].astype(f64)
    edgevec = inputs["edgevec"].astype(f64)

    DE = np.tanh(2.0 * nodevec)                     # (N, EMB)
    EE = np.tanh(2.0 * edgevec)                     # (M, EMB)
    adj = np.maximum(np.tanh(2.0 * (DE @ EE.T)), 0.0)   # (N, M)
    S = (adj > 0).astype(f64)
    Bsum = S.sum(0)
    Binv = np.where(Bsum > 0, 1.0 / np.maximum(Bsum, 1e-30), 0.0)   # (M,)
    Dsum = adj.sum(1)
    Dinv = np.where(Dsum > 0, 1.0 / np.maximum(Dsum, 1e-30), 0.0)   # (N,)

    A2 = (Dinv[:, None] * adj * Binv[None, :]) @ S.T     # (N, N): out = A2 @ hl
    # rank-R factorization A2 ~= Ur @ Vr: the SVD tail is far below the
    # bf16 noise floor (rel err 6.598e-3 at R=128 vs 6.580e-3 dense), and
    # the factored aggregation costs 48R output columns vs 24576 dense
    R = int(os.environ.get("A2_RANK", "128"))
    _u, _s, _vt = np.linalg.svd(A2)
    Ur = _u[:, :R] * _s[:R]                              # (N, R)
    Vr = _vt[:R]                                         # (R, N)
    VrT = np.zeros((NP, R), f64)
    VrT[:N] = Vr.T
    VrT = np.ascontiguousarray(VrT).reshape(NCHUNK, 128, R)
    UrT = np.zeros((R, NP), f64)
    UrT[:, :N] = Ur.T

    # channel reorder: xs rows were [xt 0:32 | state 32:96]; device buffer
    # uses [state 0:64 | ones 64 | xt 65:97] so the ones row also serves the
    # bias-augmented 1x1 conv (rhs rows 0:65)
    def reorder_aug(w, b_res, b_lin):
        # (96, C) weights -> (97, C) with the bias row at position 64
        w = np.asarray(w, f64)
        bias = np.asarray(b_res, f64) + np.asarray(b_lin, f64)
        return np.concatenate([w[32:96], bias[None, :], w[0:32]], axis=0)

    def reorder_zero(w):
        # (96, C) weights -> (97, C) with a zero row at position 64
        w = np.asarray(w, f64)
        z = np.zeros((1, w.shape[1]), f64)
        return np.concatenate([w[32:96], z, w[0:32]], axis=0)

    # gate output channels reordered to [r | z] so that r sits at partition
    # base 0 (walrus requires equal start partitions for DVE SB inputs)
    zperm = np.concatenate([np.arange(64, 128), np.arange(0, 64)])
    w_lin_g = reorder_zero(inputs["w_lin_g"])[:, zperm]       # (97, 128)
    wr_g = reorder_aug(inputs["w_res_g"], inputs["b_res_g"],
                       inputs["b_lin_g"])[:, zperm]
    w_lin_c = reorder_zero(inputs["w_lin_c"])                 # (97, 64)
    wr_c = reorder_aug(inputs["w_res_c"], inputs["b_res_c"], inputs["b_lin_c"])
    convw_aug = np.concatenate(
        [np.asarray(inputs["conv_w"], f64),
         np.asarray(inputs["conv_b"], f64)[None, :]], axis=0)  # (65, 64)

    def center_mats(gamma, C):
        g = np.asarray(gamma, f64)
        P = np.diag(g) @ (np.eye(C) - np.ones((C, C)) / C)    # cent = P @ pre
        gsq = np.where(g != 0, g * g, 1.0)
        w = (1.0 / C) / gsq                                   # var weights
        G = np.repeat(w[:, None], C, axis=1)                  # (C, C)
        return P, G

    Pg, Gg = center_mats(np.asarray(inputs["ln_g_w"], f64)[zperm], 2 * HID)
    Pc, Gc = center_mats(inputs["ln_c_w"], HID)               # (64, 64)

    def blockdiag(Ab, Bb):
        Z = np.zeros((Ab.shape[0] + Bb.shape[0], Ab.shape[1] + Bb.shape[1]), f64)
        Z[: Ab.shape[0], : Ab.shape[1]] = Ab
        Z[Ab.shape[0]:, Ab.shape[1]:] = Bb
        return Z

    Pcb = blockdiag(Pc, Pc)                                   # (128, 128)
    Gcb = blockdiag(Gc, Gc)

    consts = {
        "vrt": _to_bf16(VrT),                 # (8, 128, 128)
        "urt": _to_bf16(UrT),                 # (128, 1024)
        "wlin_g": _to_bf16(w_lin_g),          # (97, 128)
        "wres_g": _to_bf16(wr_g),             # (97, 128)
        "wlin_c": _to_bf16(w_lin_c),          # (97, 64)
        "wres_c": _to_bf16(wr_c),             # (97, 64)
        # lhsT for cent = P @ pre must be P.T (out = lhsT.T @ rhs)
        "pg": _to_bf16(Pg.T),                 # (128, 128)
        "gg": _to_bf16(Gg),                   # (128, 128) symmetric-by-rows
        "pcb": _to_bf16(Pcb.T),               # (128, 128)
        "gcb": _to_bf16(Gcb),
        "convw": _to_bf16(convw_aug),         # (65, 64)
        "ln_g_b": np.asarray(inputs["ln_g_b"], np.float32)[zperm],
        "ln_c_b": np.asarray(inputs["ln_c_b"], np.float32),
    }
    return consts


def _host_x(inputs):
    """x (B, N, F_IN, T) -> per-core (T, 32, 2048) bf16, channel-transposed."""
    x = np.asarray(inputs["x"], np.float32)
    xt = x.transpose(3, 2, 0, 1)                      # (T, F_IN, B, N)
    xp = np.zeros((T, F_IN, B, NP), np.float32)
    xp[:, :, :, :N] = xt
    shards = []
    for c in range(NCORES):
        sl = xp[:, :, c * BL:(c + 1) * BL, :].reshape(T, F_IN, NC)
        shards.append(_to_bf16(sl))
    return shards


# --------------------------------------------------------------------------
# device program
# --------------------------------------------------------------------------

def _patch_tile_drain():
    """walrus in this toolchain rejects >~2 sync-waits on one instruction;
    Tile's kernel-tail drain accumulates one wait per dangling semaphore.
    Split them across single-wait nofuse nops on the sync engine."""
    import concourse.mybir as mybir  # noqa: PLC0415
    from concourse.tile import TileContext  # noqa: PLC0415
    from concourse.vector_clock import ScopedClock  # noqa: PLC0415

    if getattr(TileContext, "_drain_waits_patched", False):
        return

    def _drain_and_barrier(self, tick_clock, wait_clock):
        collector = self.nc.sync.nop(nofuse=True, hint="tail_wait_0")
        wait_clock.add_sem_waits(
            collector.ins, ScopedClock({None: tick_clock.global_clock})
        )
        si = collector.ins.sync_info
        waits = list(si.on_wait) if si and si.on_wait else []
        if len(waits) > 1:
            collector.ins.sync_info = mybir.SyncInfo(
                on_wait=[waits[0]], on_update=list(si.on_update or [])
            )
            for k, w in enumerate(waits[1:]):
                extra = self.nc.sync.nop(nofuse=True, hint=f"tail_wait_{k + 1}")
                extra.ins.sync_info = mybir.SyncInfo(on_wait=[w], on_update=[])
        self.nc.sync.drain()
        self.nc.all_engine_barrier()
        popped = self.nc._tile_sem_poison_stack.pop()
        assert popped is self._sem_poison
        self.nc.clear_and_free_semaphores(list(self.sems.allocated().values()))
        self.nc.all_engine_barrier()

    TileContext._drain_and_barrier = _drain_and_barrier

    # Split >MAX_WAITS sem-waits on any scheduled instruction onto preceding
    # nofuse nops on the same engine (same-engine program order preserves
    # the wait semantics exactly).
    MAX_WAITS = int(os.environ.get("BASS_MAX_INST_WAITS", "1"))
    orig_lower = TileContext._lower_ordered_insts

    def _lower_ordered_insts(self, ordered):
        for bb_name, insts in ordered.items():
            out = []
            for inst in insts:
                si = inst.sync_info
                waits = list(si.on_wait) if si and si.on_wait else []
                if len(waits) > MAX_WAITS:
                    excess = waits[:-MAX_WAITS]
                    keep = waits[-MAX_WAITS:]
                    for j in range(0, len(excess), MAX_WAITS):
                        nop = mybir.InstNoOp(
                            name=self.nc.get_next_instruction_name(),
                            ins=[], outs=[], engine=inst.engine,
                        )
                        nop.bass_nofuse = True
                        nop.sync_info = mybir.SyncInfo(
                            on_wait=excess[j:j + MAX_WAITS], on_update=[]
                        )
                        out.append(nop)
                    inst.sync_info = mybir.SyncInfo(
                        on_wait=keep, on_update=list(si.on_update or [])
                    )
                out.append(inst)
            insts[:] = out
        return orig_lower(self, ordered)

    TileContext._lower_ordered_insts = _lower_ordered_insts
    TileContext._drain_waits_patched = True


def _build_bass(beta_g_nonzero, beta_c_nonzero):
    import concourse.bass as bass  # noqa: PLC0415
    import concourse.mybir as mybir  # noqa: PLC0415
    from concourse.tile import TileContext  # noqa: PLC0415

    _patch_tile_drain()

    fp32 = mybir.dt.float32
    bf16 = mybir.dt.bfloat16
    AF = mybir.ActivationFunctionType

    nc = bass.Bass()

    _reps = int(os.environ.get("WORK_REPS", "1"))
    rep_tag = nc.declare_dram_parameter("rep_tag", [1, 8 * _reps], fp32,
                                        isOutput=False)
    xT = nc.declare_dram_parameter("xT", [T, F_IN, NC], bf16, isOutput=False)
    vrt_d = nc.declare_dram_parameter("vrt", [NCHUNK, 128, 128], bf16, isOutput=False)
    urt_d = nc.declare_dram_parameter("urt", [128, NP], bf16, isOutput=False)
    wlin_g_d = nc.declare_dram_parameter("wlin_g", [97, 128], bf16, isOutput=False)
    wres_g_d = nc.declare_dram_parameter("wres_g", [97, 128], bf16, isOutput=False)
    wlin_c_d = nc.declare_dram_parameter("wlin_c", [97, 64], bf16, isOutput=False)
    wres_c_d = nc.declare_dram_parameter("wres_c", [97, 64], bf16, isOutput=False)
    pg_d = nc.declare_dram_parameter("pg", [128, 128], bf16, isOutput=False)
    gg_d = nc.declare_dram_parameter("gg", [128, 128], bf16, isOutput=False)
    pcb_d = nc.declare_dram_parameter("pcb", [128, 128], bf16, isOutput=False)
    gcb_d = nc.declare_dram_parameter("gcb", [128, 128], bf16, isOutput=False)
    convw_d = nc.declare_dram_parameter("convw", [65, 64], bf16, isOutput=False)
    out_d = nc.declare_dram_parameter("out", [T, OUT, NC], bf16, isOutput=True)

    with TileContext(nc) as tc:
        with (
            tc.tile_pool(name="const", bufs=1) as cpool,
            tc.tile_pool(name="state", bufs=1) as spool,
            tc.tile_pool(name="work", bufs=2) as wpool,
            tc.tile_pool(name="psA", bufs=1, space="PSUM") as psA,
            tc.tile_pool(name="psB", bufs=1, space="PSUM") as psB,
        ):
            # ---- constants into SBUF -------------------------------------
            vrt = cpool.tile([128, NCHUNK, 128], bf16, tag="vrt")
            for s in range(NCHUNK):
                nc.sync.dma_start(vrt[:, s, :], vrt_d[s])
            urt = cpool.tile([128, NP], bf16, tag="urt")
            nc.sync.dma_start(urt[:], urt_d[:])
            wlin_g = cpool.tile([97, 128], bf16, tag="wlg")
            nc.sync.dma_start(wlin_g[:], wlin_g_d[:])
            wres_g = cpool.tile([97, 128], bf16, tag="wrg")
            nc.sync.dma_start(wres_g[:], wres_g_d[:])
            wlin_c = cpool.tile([97, 64], bf16, tag="wlc")
            nc.sync.dma_start(wlin_c[:], wlin_c_d[:])
            wres_c = cpool.tile([97, 64], bf16, tag="wrc")
            nc.sync.dma_start(wres_c[:], wres_c_d[:])
            pg = cpool.tile([128, 128], bf16, tag="pg")
            nc.sync.dma_start(pg[:], pg_d[:])
            gg = cpool.tile([128, 128], bf16, tag="gg")
            nc.sync.dma_start(gg[:], gg_d[:])
            pcb = cpool.tile([128, 128], bf16, tag="pcb")
            nc.sync.dma_start(pcb[:], pcb_d[:])
            gcb = cpool.tile([128, 128], bf16, tag="gcb")
            nc.sync.dma_start(gcb[:], gcb_d[:])
            convw = cpool.tile([65, 64], bf16, tag="convw")
            nc.sync.dma_start(convw[:], convw_d[:])
            epsv = cpool.tile([128, 1], fp32, tag="epsv")
            nc.vector.memset(epsv[:], EPS)
            rtag = cpool.tile([1, 8 * _reps], fp32, tag="rtag")
            nc.sync.dma_start(rtag[:], rep_tag[:])

            # ---- persistent state buffers --------------------------------
            # per-(b, j) chunk tiles, 512 node cols each, so dependencies are
            # tracked at chunk granularity and the serial chain pipelines
            xsA = [[spool.tile([97, 512], bf16, tag=f"xsA{b}{j}",
                               name=f"xsA{b}{j}") for j in range(2)]
                   for b in range(BL)]
            xsB = [[spool.tile([97, 512], bf16, tag=f"xsB{b}{j}",
                               name=f"xsB{b}{j}") for j in range(2)]
                   for b in range(BL)]
            xcb = [[spool.tile([97, 512], bf16, tag=f"xc{b}{j}",
                               name=f"xcb{b}{j}") for j in range(2)]
                   for b in range(BL)]
            for b in range(BL):
                for j in range(2):
                    nc.vector.memset(xsA[b][j][64:65, :], 1.0)
                    nc.vector.memset(xsB[b][j][64:65, :], 1.0)
                    nc.vector.memset(xcb[b][j][64:65, :], 1.0)
                    nc.vector.memset(xsA[b][j][0:64, :], 0.0)   # h_0 = 0

            for b in range(BL):
                for j in range(2):
                    nc.vector.memset(xsB[b][j][0:64, :], 0.0)

            WORK_REPS = int(os.environ.get("WORK_REPS", "1"))

            for ti, t in enumerate([tt % T for tt in range(T * WORK_REPS)]):
                xs = xsA if ti % 2 == 0 else xsB
                xs_next = xsB if ti % 2 == 0 else xsA

                for b in range(BL):
                    for j in range(2):
                        nc.sync.dma_start(
                            xs[b][j][65:97, :],
                            xT[t, :, b * NP + j * 512: b * NP + (j + 1) * 512])
                        nc.sync.dma_start(
                            xcb[b][j][65:97, :],
                            xT[t, :, b * NP + j * 512: b * NP + (j + 1) * 512])

                # ---- gate: hl = xs @ Wg (nodes-on-partitions) ------------
                # per-(b, half) single-bank PSUM tiles so every chain stage
                # has tile-granular dependencies and pipelines freely
                hl = {}
                for b in range(BL):
                    for j2 in range(2):
                        ph = psA.tile([128, 512], fp32, tag=f"A{b}{j2}",
                                      name=f"ps_hl{b}{j2}_{ti}")
                        for k in range(4):
                            nc.tensor.matmul(
                                ph[:, k * 128:(k + 1) * 128],
                                xs[b][j2][0:97, k * 128:(k + 1) * 128],
                                wlin_g[:],
                            )
                        hlh = wpool.tile([128, 512], bf16, tag=f"hl_g{b}{j2}",
                                         name=f"hl{b}{j2}_{ti}")
                        nc.scalar.copy(hlh[:], ph[:])
                        hl[(b, j2)] = hlh
                pp_t = {}
                for j in range(2):
                    for b in range(BL):
                        pp = psB.tile([128, 512], fp32, tag=f"B{b}{j}",
                                      name=f"ps_pre{b}{j}_{ti}")
                        nc.tensor.matmul(pp[:], wres_g[:], xs[b][j][0:97, :],
                                         start=True, stop=False)
                        pp_t[(b, j)] = pp
                ygs = {}
                for b in range(BL):
                    yg = psA.tile([128, 512], fp32, tag=f"A{b}1",
                                  name=f"ps_yg{b}_{ti}")
                    for ls in range(NCHUNK):
                        nc.tensor.matmul(
                            yg[:, 0:128],
                            vrt[:, ls, :],
                            hl[(b, ls // 4)][:, (ls % 4) * 128:
                                             (ls % 4 + 1) * 128],
                            start=(ls == 0), stop=(ls == NCHUNK - 1),
                        )
                    ys = wpool.tile([128, 128], bf16, tag=f"yg{b}",
                                    name=f"yg{b}_{ti}")
                    nc.vector.tensor_copy(ys[:], yg[:, 0:128])
                    ygs[b] = ys

                # ---- gate: pre/agg banks interleaved with per-tile LN ----
                # emit agg bank k, then the depth-first LN chain of bank k-1:
                # PE stays continuously busy (agg of tile k overlaps Act/DVE
                # chain of tile k-1), and Act never queues behind a later
                # tile's unmet dependency.
                zr = {}
                cent_t, nm_t = {}, {}

                def emit_gate_agg(b, j):
                    nc.tensor.matmul(pp_t[(b, j)][:], ygs[b][:],
                                     urt[:, j * 512:(j + 1) * 512],
                                     start=False, stop=True)
                    return pp_t[(b, j)]

                def emit_gate_chain(b, j, pp):
                    pre = wpool.tile([128, 512], bf16, tag=f"pre_g{b}{j}",
                                     bufs=3, name=f"pre{b}{j}_{ti}")
                    nc.vector.tensor_scalar_max(pre[:], pp[:], 0.0)
                    pc = psA.tile([128, 512], fp32, tag=f"A{b}{j}",
                                  name=f"ps_cent{b}{j}_{ti}")
                    nc.tensor.matmul(pc[:], pg[:], pre[:])
                    cent_t[(b, j)] = pc
                    sq = wpool.tile([128, 512], bf16, tag=f"sq_g{b}{j}",
                                    name=f"sq{b}{j}_{ti}")
                    nc.scalar.activation(sq[:], pc[:], AF.Square)
                    pv = psB.tile([128, 512], fp32, tag=f"B{b}{j}",
                                  name=f"ps_var{b}{j}_{ti}")
                    nc.tensor.matmul(pv[:], gg[:], sq[:])
                    lnv = wpool.tile([128, 512], fp32, tag=f"lnv{b}{j}",
                                     bufs=1, name=f"lnv{b}{j}_{ti}")
                    nc.scalar.activation(lnv[:], pv[:], AF.Ln, bias=epsv[:])
                    rstd = wpool.tile([128, 512], bf16, tag=f"rstd{b}{j}",
                                      bufs=3, name=f"rstd{b}{j}_{ti}")
                    nc.scalar.activation(rstd[:], lnv[:], AF.Exp, scale=-0.5)
                    nm = wpool.tile([128, 512], bf16, tag=f"nm{b}{j}",
                                    name=f"nm{b}{j}_{ti}")
                    nc.vector.tensor_mul(nm[:], pc[:], rstd[:])
                    nm_t[(b, j)] = nm

                gtiles = [(0, 0), (1, 0), (0, 1), (1, 1)]
                prev = None
                for (b, j) in gtiles:
                    emit_gate_agg(b, j)
                    if prev is not None:
                        emit_gate_chain(*prev, pp_t[prev])
                    prev = (b, j)
                emit_gate_chain(*prev, pp_t[prev])

                # sigmoid via exp + fast reciprocal keeps ScalarE on the
                # natural_log_exp table: z = 1 / (1 + exp(-x))
                for (b, j) in gtiles:
                    ug = wpool.tile([128, 512], bf16, tag=f"ug{b}{j}",
                                    name=f"ug{b}{j}_{ti}")
                    nc.scalar.activation(ug[:], nm_t[(b, j)][:], AF.Exp,
                                         scale=-1.0)
                    dg = wpool.tile([128, 512], fp32, tag=f"dg{b}{j}",
                                    name=f"dg{b}{j}_{ti}")
                    nc.vector.tensor_scalar_add(dg[:], ug[:], 1.0)
                    zr[(b, j)] = wpool.tile([128, 512], fp32,
                                            tag=f"zr{b}{j}",
                                            name=f"zr{b}{j}_{ti}")
                    nc.vector.reciprocal_approx_fast(zr[(b, j)][:], dg[:])
                    nc.vector.tensor_mul(xcb[b][j][0:64, :],
                                         zr[(b, j)][0:64, :],
                                         xs[b][j][0:64, :])

                # ---- candidate: hlc half 0, then residual matmuls and
                # the first half of the y-stage fill PE while half 1's gate
                # tails are still finishing on DVE --------------------------
                hlc = {}

                def emit_hlc_half(j2):
                    ph = psA.tile([128, 512], fp32, tag=f"A0{j2}",
                                  name=f"ps_hlc{j2}_{ti}")
                    for k in range(4):
                        for b in range(BL):
                            nc.tensor.matmul(
                                ph[:, k * 128 + b * 64: k * 128 + (b + 1) * 64],
                                xcb[b][j2][0:97, k * 128:(k + 1) * 128],
                                wlin_c[:],
                            )
                    hh = wpool.tile([128, 512], bf16, tag=f"hl_c{j2}",
                                    name=f"hlc{j2}_{ti}")
                    nc.scalar.copy(hh[:], ph[:])
                    hlc[j2] = hh

                emit_hlc_half(0)
                ppc = {}
                for j in range(2):
                    pp = psB.tile([128, 512], fp32, tag=f"B0{j}",
                                  name=f"ps_prec{j}_{ti}")
                    for b in range(BL):
                        nc.tensor.matmul(
                            pp[b * 64:(b + 1) * 64, :],
                            wres_c[:],
                            xcb[b][j][0:97, :],
                            start=True, stop=False,
                            tile_position=(0, b * 64),
                        )
                    ppc[j] = pp
                yc = psA.tile([128, 512], fp32, tag="A00",
                              name=f"ps_yc_{ti}")
                for ls in range(4):
                    nc.tensor.matmul(
                        yc[:, 0:128],
                        vrt[:, ls, :],
                        hlc[0][:, (ls % 4) * 128:(ls % 4 + 1) * 128],
                        start=(ls == 0), stop=False,
                    )
                emit_hlc_half(1)
                for ls in range(4, NCHUNK):
                    nc.tensor.matmul(
                        yc[:, 0:128],
                        vrt[:, ls, :],
                        hlc[1][:, (ls % 4) * 128:(ls % 4 + 1) * 128],
                        start=False, stop=(ls == NCHUNK - 1),
                    )
                ycs = wpool.tile([128, 128], bf16, tag="yc",
                                 name=f"yc_{ti}")
                nc.vector.tensor_copy(ycs[:], yc[:, 0:128])

                def emit_cand_agg(j):
                    nc.tensor.matmul(ppc[j][:], ycs[:],
                                     urt[:, j * 512:(j + 1) * 512],
                                     start=False, stop=True)
                    return ppc[j]

                hcs = {}
                nmc_t = {}

                def emit_cand_chain(j, pp):
                    prec = wpool.tile([128, 512], bf16, tag=f"pre_c{j}",
                                      name=f"prec{j}_{ti}")
                    nc.vector.tensor_scalar_max(prec[:], pp[:], 0.0)
                    pc = psA.tile([128, 512], fp32, tag=f"A1{j}",
                                  name=f"ps_centc{j}_{ti}")
                    nc.tensor.matmul(pc[:], pcb[:], prec[:])
                    sqc = wpool.tile([128, 512], bf16, tag=f"sq_c{j}",
                                     name=f"sqc{j}_{ti}")
                    nc.scalar.activation(sqc[:], pc[:], AF.Square)
                    pv = psB.tile([128, 512], fp32, tag=f"B1{j}",
                                  name=f"ps_varc{j}_{ti}")
                    nc.tensor.matmul(pv[:], gcb[:], sqc[:])
                    lnvc = wpool.tile([128, 512], fp32, tag=f"lnvc{j}",
                                      bufs=1, name=f"lnvc{j}_{ti}")
                    nc.scalar.activation(lnvc[:], pv[:], AF.Ln, bias=epsv[:])
                    rstdc = wpool.tile([128, 512], bf16, tag=f"rstdc{j}",
                                       name=f"rstdc{j}_{ti}")
                    nc.scalar.activation(rstdc[:], lnvc[:], AF.Exp,
                                         scale=-0.5)
                    nmc = wpool.tile([128, 512], bf16, tag=f"nmc{j}",
                                     name=f"nmc{j}_{ti}")
                    nc.vector.tensor_mul(nmc[:], pc[:], rstdc[:])
                    nmc_t[j] = nmc

                pp0 = emit_cand_agg(0)
                pp1 = emit_cand_agg(1)
                emit_cand_chain(0, pp0)
                emit_cand_chain(1, pp1)
                # tanh via exp + fast reciprocal (same act table):
                # tanh(x) = 2 / (1 + exp(-2x)) - 1; the (b, j) update chain
                # is emitted right after its j's tanh so DVE/PE start the
                # state update while ScalarE is still on the other half
                for j in range(2):
                    wc = wpool.tile([128, 512], bf16, tag=f"wc{j}",
                                    name=f"wc{j}_{ti}")
                    nc.scalar.activation(wc[:], nmc_t[j][:], AF.Exp,
                                         scale=-2.0)
                    dc = wpool.tile([128, 512], fp32, tag=f"dc{j}",
                                    name=f"dc{j}_{ti}")
                    nc.vector.tensor_scalar_add(dc[:], wc[:], 1.0)
                    rc = wpool.tile([128, 512], fp32, tag=f"rc{j}",
                                    name=f"rc{j}_{ti}")
                    nc.vector.reciprocal_approx_fast(rc[:], dc[:])
                    hcs[j] = wpool.tile([128, 512], bf16, tag=f"hcs{j}",
                                        name=f"hcs{j}_{ti}")
                    nc.vector.tensor_scalar(hcs[j][:], rc[:], 2.0, 1.0,
                                            op0=mybir.AluOpType.mult,
                                            op1=mybir.AluOpType.subtract)
                    for b in range(BL):
                        difb = wpool.tile([128, 512], bf16, tag=f"difb{b}{j}",
                                          name=f"difb{b}{j}_{ti}")
                        if b == 0:
                            nc.vector.tensor_sub(difb[64:128, :],
                                                 hcs[j][0:64, :],
                                                 xs[b][j][0:64, :])
                        else:
                            hc = wpool.tile([64, 512], bf16, tag=f"hc{b}{j}",
                                            name=f"hc{b}{j}_{ti}")
                            nc.vector.tensor_copy(hc[:], hcs[j][64:128, :])
                            nc.vector.tensor_sub(difb[64:128, :], hc[:],
                                                 xs[b][j][0:64, :])
                        zd = wpool.tile([64, 512], bf16, tag=f"zd{b}{j}",
                                        name=f"zd{b}{j}_{ti}")
                        nc.vector.tensor_mul(zd[:], zr[(b, j)][64:128, :],
                                             difb[64:128, :])
                        nc.vector.tensor_add(xs_next[b][j][0:64, :],
                                             xs[b][j][0:64, :], zd[:])
                        po = psA.tile([128, 512], fp32, tag=f"A{b}{j}",
                                      name=f"ps_out{b}{j}_{ti}")
                        nc.tensor.matmul(po[0:64, :], convw[:],
                                         xs_next[b][j][0:65, :])
                        otb = wpool.tile([64, 512], bf16, tag=f"ot{b}{j}",
                                         name=f"ot{b}{j}_{ti}")
                        nc.scalar.copy(otb[:], po[0:64, :])
                        nc.sync.dma_start(
                            out_d[t, :, b * NP + j * 512:
                                  b * NP + (j + 1) * 512],
                            otb[:])

    # populate .instr bytes for extended-inst InstISA subclasses (the
    # custom-DVE reciprocal) — without this walrus sees empty .instr and
    # fails with "ISA wrong length"
    mybir.codegen_inst_isa_subclasses(nc)
    return nc



# --------------------------------------------------------------------------
# entry point
# --------------------------------------------------------------------------

def kernel(**inputs):
    from concourse.bass_utils import run_bass_kernel_spmd  # noqa: PLC0415

    consts = _host_prep(inputs)
    xshards = _host_x(inputs)

    beta_g_nonzero = bool(np.any(consts["ln_g_b"] != 0))
    beta_c_nonzero = bool(np.any(consts["ln_c_b"] != 0))
    assert not beta_g_nonzero and not beta_c_nonzero, "beta path not wired yet"

    nc = _build_bass(beta_g_nonzero, beta_c_nonzero)

    base = {k: np.asarray(v) for k, v in consts.items()
            if k not in ("ln_g_b", "ln_c_b")}
    in_maps = []
    reps = int(os.environ.get("WORK_REPS", "1"))
    for c in range(NCORES):
        m = dict(base)
        m["xT"] = xshards[c]
        m["rep_tag"] = np.zeros((1, 8 * reps), np.float32)
        in_maps.append(m)

    res = run_bass_kernel_spmd(nc, in_maps, core_ids=list(range(NCORES)))
    outs = []
    for c in range(NCORES):
        o = np.asarray(res.results[c]["out"]).astype(np.float32)
        o = o.reshape(T, OUT, BL, NP)[:, :, :, :N]   # (T, 64, 2, 1000)
        outs.append(o.transpose(2, 3, 1, 0))         # (2, 1000, 64, 12)
    full = np.concatenate(outs, axis=0).astype(np.float32)
    return full


if __name__ == "__main__":
    print("kernel module loaded")

